# revision 67
# baseline (speedup 1.0000x reference)
"""Batched 20x20 SPD covariance-matrix inversion on 8 Trainium2 NeuronCores.

For each of 131072 batches: build C = exp(-1.5 * pairwise_dist(pos)) + 0.01*I
from 20 2-D points, return C^{-1}.

Strategy (per core, data-parallel over batch):
 - batch-major layout: each of 128 SBUF partitions holds M matrices' full
   20x20 (400 fp32) in the free dim, in variable-size chunks (half-size
   ramp chunks at both ends shorten pipeline fill/drain).
 - symmetric Gauss-Jordan sweep over 20 pivots, upper-triangle only
   (4-rect cover), skipping the pivot row/column (set exactly by copies).
 - work split across THREE engines, statically balanced:
     DVE  (0.96 GHz, 1x fp32 TT)     ~65% of the rank-1 update passes
     Pool (GPSIMD TT @ 0.42 eff)     ~35% of the update passes
     ACT  (1.2 GHz)                  squares/sqrt/exp, gathers, copies,
                                     mirror; all single-tensor traffic
   Assignment alternates by (pivot+chunk) parity so outer->sub chains stay
   within one engine while two in-flight chunks complement each other.
 - rank-1 tmp buffers for rects 1-3 live inside A's dead lower triangle
   (overwritten later by the mirror); only rect 0 needs a scratch tile.
 - final negate folded into the last pivot (sub pass emits tmp - A).
 - rolling software pipeline: chunk c sweeps pivots 0-9 while chunk c-1
   sweeps pivots 10-19, chunk c-2 mirrors + DMAs out, and chunk c+1
   builds its covariance (A pool bufs=3).
"""

import numpy as np

import concourse.bass as bass  # noqa: F401  (registers engine APIs)
import concourse.tile as tile
from concourse import bacc, mybir
from concourse.bass_utils import run_bass_kernel_spmd

N = 20                  # matrix dim
D = 2                   # coord dim
PHI = 1.5
TAU = 0.01
P = 128                 # SBUF partitions
N_CORES = 8
B_TOTAL = 131072
B_CORE = B_TOTAL // N_CORES   # 16384

F32 = mybir.dt.float32
AF = mybir.ActivationFunctionType
OP = mybir.AluOpType

# Upper-triangle rectangle cover: rows [r0,r1) x cols [r0,N)
RECTS = [(0, 5), (5, 10), (10, 15), (15, 20)]
_upd_sizes = [(r1 - r0) * (N - r0) for r0, r1 in RECTS]

# Per-rect engine assignment for the rank-1 update passes, by parity of
# (pivot + chunk): keeps outer->sub chains within one engine while the two
# interleaved chunks use complementary assignments each step.
# 'd' = DVE, 'p' = Pool/GPSIMD.  (outer_engine, sub_engine) per rect.
# NOTE: only plain tensor_tensor ops lower to hardware for these access
# patterns (scalar_tensor_tensor requires fully-contiguous operands).
UPD_ENG_PAR = [
    [("p", "p"), ("d", "d"), ("d", "d"), ("d", "d")],   # parity 0
    [("d", "d"), ("p", "p"), ("d", "d"), ("d", "d")],   # parity 1
]
CR_ENG_PAR = ["d", "d"]
# Rects 1-3 keep their outer-product tmp inside A's dead lower triangle:
# rect index -> (rows, cols) of a lower-triangle scratch block of equal shape
TMP_SLOT = {1: (15, 0), 2: (10, 0), 3: (5, 0)}
# build passes: (dx_eng, dy_eng, add_eng) per rect; squares/sqrt/exp on ACT
BUILD_ENG = [("d", "p", "d"), ("d", "p", "p"), ("d", "d", "d"), ("d", "d", "p")]
MIRROR_CYCLE = ("a",)


def emit_kernel(tc, pos_ap, out_ap, b_core, m_chunk):
    """Emit the per-core program. pos: [b_core, 40] f32, out: [b_core, 400] f32."""
    nc = tc.nc
    rows = b_core // P          # matrices per partition
    assert b_core == P * rows
    MB = m_chunk                # max chunk size (tile allocation size)
    # half-size ramp chunks at both ends shorten pipeline fill/drain
    if rows > MB and (rows - MB) % MB == 0 and MB % 2 == 0:
        sizes = [MB // 2] + [MB] * ((rows - MB) // MB) + [MB // 2]
    else:
        assert rows % MB == 0
        sizes = [MB] * (rows // MB)
    offs = [sum(sizes[:i]) for i in range(len(sizes))]
    chunks = len(sizes)

    def eng(c):
        return {"d": nc.vector, "p": nc.gpsimd, "a": nc.scalar}[c]

    pos_f = pos_ap.rearrange("(p r) f -> p (r f)", p=P)
    out_f = out_ap.rearrange("(p r) f -> p (r f)", p=P)

    max_rect = max(_upd_sizes)

    with (
        tc.tile_pool(name="pos", bufs=1) as pos_pool,
        tc.tile_pool(name="A", bufs=3) as a_pool,
        tc.tile_pool(name="tp", bufs=2) as tp_pool,
        tc.tile_pool(name="dy0", bufs=1) as dy0_pool,
        tc.tile_pool(name="small", bufs=2) as small_pool,
    ):
        A4s = {}
        posvs = {}
        ptiles = {}
        badds = {}

        # persistent zeros used to blank the pivot slot of the gathered column
        zt, _zt_free = tc.tile([P, MB], F32, name="zt")
        nc.gpsimd.memset(zt[:, :], 0.0)

        def build_start(c):
            M = sizes[c]
            pos_t = pos_pool.tile([P, MB * N * D], F32)
            nc.sync.dma_start(
                pos_t[:, 0 : M * N * D],
                pos_f[:, offs[c] * N * D : (offs[c] + M) * N * D],
            )
            posvs[c] = pos_t[:, 0 : M * N * D].rearrange(
                "p (m i d) -> p m i d", m=M, i=N
            )
            A = a_pool.tile([P, MB * N * N], F32)
            A4 = A[:, 0 : M * N * N].rearrange("p (m i j) -> p m i j", m=M, i=N)
            A4s[c] = (A, A4)

        def build_rect(c, ri):
            M = sizes[c]
            posv = posvs[c]
            _, A4 = A4s[c]
            (r0, r1), (e_dx, e_dy, e_add) = RECTS[ri], BUILD_ENG[ri]
            nr, ncl = r1 - r0, N - r0
            reg = A4[:, :, r0:r1, r0:]
            xi = posv[:, :, r0:r1, 0].unsqueeze(3).broadcast_to([P, M, nr, ncl])
            xj = posv[:, :, r0:, 0].unsqueeze(2).broadcast_to([P, M, nr, ncl])
            yi = posv[:, :, r0:r1, 1].unsqueeze(3).broadcast_to([P, M, nr, ncl])
            yj = posv[:, :, r0:, 1].unsqueeze(2).broadcast_to([P, M, nr, ncl])
            if ri in TMP_SLOT:
                # chunk's own update-tmp slots are free during its build
                tr, tc_ = TMP_SLOT[ri]
                dyv = A4[:, :, tr : tr + nr, tc_ : tc_ + ncl]
            else:
                dy = dy0_pool.tile([P, MB * max_rect], F32, tag="dy0")
                dyv = dy[:, 0 : M * nr * ncl].rearrange(
                    "p (m i j) -> p m i j", m=M, i=nr
                )
            eng(e_dx).tensor_sub(reg, xi, xj)
            eng(e_dy).tensor_sub(dyv, yi, yj)
            nc.scalar.square(reg, reg)
            nc.scalar.square(dyv, dyv)
            badds[(c, ri)] = (reg, dyv, e_add)

        def build_add(c, ri):
            # emitted steps after the squares so the add never head-of-line
            # blocks its engine waiting on ACT
            reg, dyv, e_add = badds.pop((c, ri))
            eng(e_add).tensor_add(reg, reg, dyv)

        def build_end(c):
            M = sizes[c]
            A, A4 = A4s[c]
            # batched sqrt then exp: no ACT table holds both funcs, so
            # grouping them gives 2 table loads per chunk instead of 2/rect
            for r0, r1 in RECTS:
                nc.scalar.sqrt(A4[:, :, r0:r1, r0:], A4[:, :, r0:r1, r0:])
            for r0, r1 in RECTS:
                nc.scalar.activation(
                    A4[:, :, r0:r1, r0:], A4[:, :, r0:r1, r0:], AF.Exp,
                    scale=-PHI,
                )
            Av = A[:, 0 : M * N * N].rearrange("p (m x) -> p m x", m=M)
            diag = Av[:, :, 0 : N * N : N + 1]
            nc.vector.tensor_scalar_add(diag, diag, TAU)
            posvs.pop(c)

        def pivot_head(c, k):
            M = sizes[c]
            _, A4 = A4s[c]
            cK = small_pool.tile([P, MB * N], F32, tag="c")
            crK = small_pool.tile([P, MB * N], F32, tag="cr")
            rK_t = small_pool.tile([P, MB], F32, tag="r")
            rK = rK_t[:, 0:M]  # AP view of the per-chunk prefix
            c3 = cK[:, 0 : M * N].rearrange("p (m i) -> p m i", m=M)
            cr3 = crK[:, 0 : M * N].rearrange("p (m i) -> p m i", m=M)
            ptiles[c] = (c3, cr3)

            # reciprocal straight off the diagonal (no gather dependency)
            nc.vector.reciprocal(rK, A4[:, :, k, k])
            # gather pivot column from upper storage (ACT engine); slot k <- 0
            if k:
                nc.scalar.copy(c3[:, :, :k], A4[:, :, :k, k])
            if k < N - 1:
                nc.scalar.copy(c3[:, :, k + 1 :], A4[:, :, k, k + 1 :])
            nc.scalar.copy(c3[:, :, k], zt[:, 0:M])
            # diag <- -r (+r on the last pivot: updates never touch the
            # pivot row/col, and the last pivot's update negates the rest)
            last = k == N - 1
            nc.scalar.activation(
                A4[:, :, k, k], rK, AF.Copy, scale=1.0 if last else -1.0
            )
            rb = rK.unsqueeze(2).broadcast_to([P, M, N])
            eng(CR_ENG_PAR[(k + c) % 2]).tensor_mul(cr3, c3, rb)
            # pivot row/col (upper parts) <- cr (-cr on the last pivot)
            if k:
                nc.scalar.activation(
                    A4[:, :, :k, k], cr3[:, :, :k], AF.Copy,
                    scale=-1.0 if last else 1.0,
                )
            if k < N - 1:
                nc.scalar.copy(A4[:, :, k, k + 1 :], cr3[:, :, k + 1 :])

        def pivot_body(c, k):
            M = sizes[c]
            _, A4 = A4s[c]
            c3, cr3 = ptiles[c]
            last = k == N - 1
            # rank-1 update of the upper triangle (rect cover), skipping the
            # pivot row and column (they are set exactly by pivot_head);
            # returns emission closures so the scheduler can interleave the
            # two in-flight chunks' pieces
            pieces = []
            t_p = tp_pool.tile([P, MB * max_rect], F32, tag="tp")
            upd_eng = UPD_ENG_PAR[(k + c) % 2]
            for ri, ((r0, r1), (e_out, e_sub)) in enumerate(zip(RECTS, upd_eng)):
                rowg = (
                    [(r0, k), (k + 1, r1)] if r0 <= k < r1 else [(r0, r1)]
                )
                for a, b in rowg:
                    if a >= b:
                        continue
                    # skip the pivot column only when the row group is tall
                    # enough to amortize the extra op (c3[k]=0 keeps an
                    # unsplit region correct); the last pivot's split is free
                    if r0 <= k and (b - a >= 3 or last):
                        colg = [(r0, k), (k + 1, N)]
                    else:
                        colg = [(r0, N)]
                    for ca, cb_ in colg:
                        if ca >= cb_:
                            continue
                        nr, ncl = b - a, cb_ - ca
                        if ri in TMP_SLOT:
                            tr, tc_ = TMP_SLOT[ri]
                            tv = A4[
                                :, :,
                                tr + (a - r0) : tr + (b - r0),
                                tc_ + (ca - r0) : tc_ + (cb_ - r0),
                            ]
                        else:
                            tv = t_p[:, 0 : M * (r1 - r0) * (N - r0)].rearrange(
                                "p (m i j) -> p m i j", m=M, i=r1 - r0
                            )[:, :, a - r0 : b - r0, ca - r0 : cb_ - r0]
                        cb = (
                            c3[:, :, a:b]
                            .unsqueeze(3)
                            .broadcast_to([P, M, nr, ncl])
                        )
                        crb = (
                            cr3[:, :, ca:cb_]
                            .unsqueeze(2)
                            .broadcast_to([P, M, nr, ncl])
                        )
                        reg = A4[:, :, a:b, ca:cb_]

                        def emit(tv=tv, cb=cb, crb=crb, reg=reg,
                                 e_out=e_out, e_sub=e_sub, last=last):
                            eng(e_out).tensor_mul(tv, cb, crb)
                            if last:
                                # tmp - A = -(A - tmp): folds the negate in
                                eng(e_sub).tensor_sub(reg, tv, reg)
                            else:
                                eng(e_sub).tensor_sub(reg, reg, tv)

                        pieces.append(emit)
            return pieces

        def mirror_cols(c, cols):
            _, A4 = A4s[c]
            for i in cols:
                e = eng(MIRROR_CYCLE[i % len(MIRROR_CYCLE)])
                if e is nc.scalar:
                    e.copy(A4[:, :, i + 1 :, i], A4[:, :, i, i + 1 :])
                else:
                    e.tensor_copy(A4[:, :, i + 1 :, i], A4[:, :, i, i + 1 :])

        def dma_out(c):
            M = sizes[c]
            A, _ = A4s.pop(c)
            nc.sync.dma_start(
                out_f[:, offs[c] * N * N : (offs[c] + M) * N * N],
                A[:, 0 : M * N * N],
            )

        # ---- rolling pipeline ----
        # phase f: sweeps pivots 0..9 of chunk f and 10..19 of chunk f-1;
        # chunk f-2 is mirrored + DMA'd out in the early steps, then chunk
        # f+1 builds mid-phase (reusing f-2's A slot; pool bufs=3).
        HALF = N // 2
        n_rects = len(RECTS)
        build_start(0)
        for ri in range(n_rects):
            build_rect(0, ri)
        for ri in range(n_rects):
            build_add(0, ri)
        build_end(0)
        for f in range(chunks + 2):
            cur = f if f < chunks else None        # sweeping pivots 0..9
            prev = f - 1 if 0 <= f - 1 < chunks else None   # pivots 10..19
            fin = f - 2 if 0 <= f - 2 < chunks else None    # mirror + DMA
            nxt = f + 1 if f + 1 < chunks else None
            for k in range(HALF):
                if fin is not None:
                    if k == 0:
                        mirror_cols(fin, range(0, N // 2))
                    elif k == 1:
                        mirror_cols(fin, range(N // 2, N - 1))
                    elif k == 2:
                        dma_out(fin)
                if nxt is not None:
                    if k == 2:
                        build_start(nxt)
                        for ri in range(n_rects):
                            build_rect(nxt, ri)
                    elif k == 3:
                        for ri in range(n_rects):
                            build_add(nxt, ri)
                    elif k == 4:
                        build_end(nxt)
                if cur is not None:
                    pivot_head(cur, k)
                if prev is not None:
                    pivot_head(prev, HALF + k)
                bod_a = pivot_body(cur, k) if cur is not None else []
                bod_b = pivot_body(prev, HALF + k) if prev is not None else []
                for e in bod_a:
                    e()
                for e in bod_b:
                    e()
        _zt_free()


_CACHE = {}


def build_nc(b_core=B_CORE, m_chunk=32, num_devices=N_CORES):
    key = (b_core, m_chunk, num_devices)
    if key in _CACHE:
        return _CACHE[key]
    nc = bacc.Bacc(
        "TRN2", target_bir_lowering=False, debug=False, num_devices=num_devices
    )
    pos_d = nc.dram_tensor("pos", [b_core, N * D], F32, kind="ExternalInput")
    out_d = nc.dram_tensor("out", [b_core, N * N], F32, kind="ExternalOutput")
    with tile.TileContext(nc) as tc:
        emit_kernel(tc, pos_d.ap(), out_d.ap(), b_core, m_chunk)
    nc.compile()
    _CACHE[key] = nc
    return nc


def run(pos_full, b_core=B_CORE, m_chunk=32, n_cores=N_CORES, **kw):
    """pos_full: [n_cores*b_core, 20, 2] f32 -> [n_cores*b_core, 20, 20] f32."""
    nc = build_nc(b_core, m_chunk, n_cores)
    flat = np.ascontiguousarray(
        np.asarray(pos_full, dtype=np.float32).reshape(-1, N * D)
    )
    in_maps = [
        {"pos": flat[i * b_core : (i + 1) * b_core]} for i in range(n_cores)
    ]
    res = run_bass_kernel_spmd(nc, in_maps, core_ids=list(range(n_cores)), **kw)
    out = np.concatenate([r["out"] for r in res.results], axis=0)
    return out.reshape(-1, N, N), res


def kernel(neighbor_positions, edge_list=None):
    out, _ = run(neighbor_positions)
    return out


# revision 80
# speedup vs baseline: 1.0428x; 1.0428x over previous
"""Batched 20x20 SPD covariance-matrix inversion on 8 Trainium2 NeuronCores.

For each of 131072 batches: build C = exp(-1.5 * pairwise_dist(pos)) + 0.01*I
from 20 2-D points, return C^{-1}.

Strategy (per core, data-parallel over batch):
 - batch-major layout: each of 128 SBUF partitions holds M matrices' full
   20x20 (400 fp32) in the free dim, in variable-size chunks (half-size
   ramp chunks at both ends shorten pipeline fill/drain).
 - symmetric Gauss-Jordan sweep over 20 pivots, upper-triangle only
   (height-4 5-rect cover), skipping the pivot row/column (set by copies).
 - work split across THREE engines, statically balanced:
     DVE  (0.96 GHz, 1x fp32 TT)     ~65% of the rank-1 update passes
     Pool (GPSIMD TT @ 0.42 eff)     ~35% of the update passes
     ACT  (1.2 GHz)                  squares/sqrt/exp, gathers, copies,
                                     mirror; all single-tensor traffic
   Assignment alternates by (pivot+chunk) parity so outer->sub chains stay
   within one engine while two in-flight chunks complement each other.
 - rank-1 tmp buffers for rects 1-4 live inside A's dead lower triangle
   (overwritten later by the mirror); only rect 0 needs a scratch tile.
 - final negate folded into the last pivot (sub pass emits tmp - A).
 - rolling software pipeline: chunk c sweeps pivots 0-9 while chunk c-1
   sweeps pivots 10-19, chunk c-2 mirrors + DMAs out, and chunk c+1
   builds its covariance (A pool bufs=3).
"""

import numpy as np

import concourse.bass as bass  # noqa: F401  (registers engine APIs)
import concourse.tile as tile
from concourse import bacc, mybir
from concourse.bass_utils import run_bass_kernel_spmd

N = 20                  # matrix dim
D = 2                   # coord dim
PHI = 1.5
TAU = 0.01
P = 128                 # SBUF partitions
N_CORES = 8
B_TOTAL = 131072
B_CORE = B_TOTAL // N_CORES   # 16384

F32 = mybir.dt.float32
AF = mybir.ActivationFunctionType
OP = mybir.AluOpType

# Upper-triangle rectangle cover: rows [r0,r1) x cols [r0,N)
RECTS = [(0, 4), (4, 8), (8, 12), (12, 16), (16, 20)]
_upd_sizes = [(r1 - r0) * (N - r0) for r0, r1 in RECTS]

# Per-rect engine assignment for the rank-1 update passes, by parity of
# (pivot + chunk): keeps outer->sub chains within one engine while the two
# interleaved chunks use complementary assignments each step.
# 'd' = DVE, 'p' = Pool/GPSIMD.  (outer_engine, sub_engine) per rect.
# NOTE: only plain tensor_tensor ops lower to hardware for these access
# patterns (scalar_tensor_tensor requires fully-contiguous operands).
UPD_ENG_PAR = [
    [("p", "p"), ("d", "d"), ("d", "d"), ("d", "d"), ("d", "d")],   # parity 0
    [("d", "d"), ("p", "p"), ("d", "d"), ("d", "d"), ("p", "p")],   # parity 1
]
CR_ENG_PAR = ["d", "d"]
# Rects 1-3 keep their outer-product tmp inside A's dead lower triangle:
# rect index -> (rows, cols) of a lower-triangle scratch block of equal shape
TMP_SLOT = {1: (16, 0), 2: (12, 0), 3: (8, 0), 4: (4, 0)}
# build passes: (dx_eng, dy_eng, add_eng) per rect; squares/sqrt/exp on ACT
BUILD_ENG = [("d", "p", "p"), ("d", "p", "d"), ("d", "d", "d"), ("d", "d", "d"), ("d", "d", "p")]
MIRROR_CYCLE = ("a",)


def emit_kernel(tc, pos_ap, out_ap, b_core, m_chunk):
    """Emit the per-core program. pos: [b_core, 40] f32, out: [b_core, 400] f32."""
    nc = tc.nc
    rows = b_core // P          # matrices per partition
    assert b_core == P * rows
    MB = m_chunk                # max chunk size (tile allocation size)
    # half-size ramp chunks at both ends shorten pipeline fill/drain
    if rows > MB and (rows - MB) % MB == 0 and MB % 2 == 0:
        sizes = [MB // 2] + [MB] * ((rows - MB) // MB) + [MB // 2]
    else:
        assert rows % MB == 0
        sizes = [MB] * (rows // MB)
    offs = [sum(sizes[:i]) for i in range(len(sizes))]
    chunks = len(sizes)

    def eng(c):
        return {"d": nc.vector, "p": nc.gpsimd, "a": nc.scalar}[c]

    pos_f = pos_ap.rearrange("(p r) f -> p (r f)", p=P)
    out_f = out_ap.rearrange("(p r) f -> p (r f)", p=P)

    max_rect = max(_upd_sizes)

    with (
        tc.tile_pool(name="pos", bufs=1) as pos_pool,
        tc.tile_pool(name="A", bufs=3) as a_pool,
        tc.tile_pool(name="tp", bufs=2) as tp_pool,
        tc.tile_pool(name="dy0", bufs=1) as dy0_pool,
        tc.tile_pool(name="small", bufs=2) as small_pool,
    ):
        A4s = {}
        posvs = {}
        ptiles = {}
        badds = {}

        # persistent zeros used to blank the pivot slot of the gathered column
        zt, _zt_free = tc.tile([P, MB], F32, name="zt")
        nc.gpsimd.memset(zt[:, :], 0.0)

        def build_start(c):
            M = sizes[c]
            pos_t = pos_pool.tile([P, MB * N * D], F32)
            nc.sync.dma_start(
                pos_t[:, 0 : M * N * D],
                pos_f[:, offs[c] * N * D : (offs[c] + M) * N * D],
            )
            posvs[c] = pos_t[:, 0 : M * N * D].rearrange(
                "p (m i d) -> p m i d", m=M, i=N
            )
            A = a_pool.tile([P, MB * N * N], F32)
            A4 = A[:, 0 : M * N * N].rearrange("p (m i j) -> p m i j", m=M, i=N)
            A4s[c] = (A, A4)

        def build_rect(c, ri):
            M = sizes[c]
            posv = posvs[c]
            _, A4 = A4s[c]
            (r0, r1), (e_dx, e_dy, e_add) = RECTS[ri], BUILD_ENG[ri]
            nr, ncl = r1 - r0, N - r0
            reg = A4[:, :, r0:r1, r0:]
            xi = posv[:, :, r0:r1, 0].unsqueeze(3).broadcast_to([P, M, nr, ncl])
            xj = posv[:, :, r0:, 0].unsqueeze(2).broadcast_to([P, M, nr, ncl])
            yi = posv[:, :, r0:r1, 1].unsqueeze(3).broadcast_to([P, M, nr, ncl])
            yj = posv[:, :, r0:, 1].unsqueeze(2).broadcast_to([P, M, nr, ncl])
            if ri in TMP_SLOT:
                # chunk's own update-tmp slots are free during its build
                tr, tc_ = TMP_SLOT[ri]
                dyv = A4[:, :, tr : tr + nr, tc_ : tc_ + ncl]
            else:
                dy = dy0_pool.tile([P, MB * max_rect], F32, tag="dy0")
                dyv = dy[:, 0 : M * nr * ncl].rearrange(
                    "p (m i j) -> p m i j", m=M, i=nr
                )
            eng(e_dx).tensor_sub(reg, xi, xj)
            eng(e_dy).tensor_sub(dyv, yi, yj)
            nc.scalar.square(reg, reg)
            nc.scalar.square(dyv, dyv)
            badds[(c, ri)] = (reg, dyv, e_add)

        def build_add(c, ri):
            # emitted steps after the squares so the add never head-of-line
            # blocks its engine waiting on ACT
            reg, dyv, e_add = badds.pop((c, ri))
            eng(e_add).tensor_add(reg, reg, dyv)

        def build_end(c):
            M = sizes[c]
            A, A4 = A4s[c]
            # batched sqrt then exp: no ACT table holds both funcs, so
            # grouping them gives 2 table loads per chunk instead of 2/rect
            for r0, r1 in RECTS:
                nc.scalar.sqrt(A4[:, :, r0:r1, r0:], A4[:, :, r0:r1, r0:])
            for r0, r1 in RECTS:
                nc.scalar.activation(
                    A4[:, :, r0:r1, r0:], A4[:, :, r0:r1, r0:], AF.Exp,
                    scale=-PHI,
                )
            Av = A[:, 0 : M * N * N].rearrange("p (m x) -> p m x", m=M)
            diag = Av[:, :, 0 : N * N : N + 1]
            nc.vector.tensor_scalar_add(diag, diag, TAU)
            posvs.pop(c)

        def pivot_head(c, k):
            M = sizes[c]
            _, A4 = A4s[c]
            cK = small_pool.tile([P, MB * N], F32, tag="c")
            crK = small_pool.tile([P, MB * N], F32, tag="cr")
            rK_t = small_pool.tile([P, MB], F32, tag="r")
            rK = rK_t[:, 0:M]  # AP view of the per-chunk prefix
            c3 = cK[:, 0 : M * N].rearrange("p (m i) -> p m i", m=M)
            cr3 = crK[:, 0 : M * N].rearrange("p (m i) -> p m i", m=M)
            ptiles[c] = (c3, cr3)

            # reciprocal straight off the diagonal (no gather dependency)
            nc.vector.reciprocal(rK, A4[:, :, k, k])
            # gather pivot column from upper storage (ACT engine); slot k <- 0
            if k:
                nc.scalar.copy(c3[:, :, :k], A4[:, :, :k, k])
            if k < N - 1:
                nc.scalar.copy(c3[:, :, k + 1 :], A4[:, :, k, k + 1 :])
            nc.scalar.copy(c3[:, :, k], zt[:, 0:M])
            # diag <- -r (+r on the last pivot: updates never touch the
            # pivot row/col, and the last pivot's update negates the rest)
            last = k == N - 1
            nc.scalar.activation(
                A4[:, :, k, k], rK, AF.Copy, scale=1.0 if last else -1.0
            )
            rb = rK.unsqueeze(2).broadcast_to([P, M, N])
            eng(CR_ENG_PAR[(k + c) % 2]).tensor_mul(cr3, c3, rb)
            # pivot row/col (upper parts) <- cr (-cr on the last pivot)
            if k:
                nc.scalar.activation(
                    A4[:, :, :k, k], cr3[:, :, :k], AF.Copy,
                    scale=-1.0 if last else 1.0,
                )
            if k < N - 1:
                nc.scalar.copy(A4[:, :, k, k + 1 :], cr3[:, :, k + 1 :])

        def pivot_body(c, k):
            M = sizes[c]
            _, A4 = A4s[c]
            c3, cr3 = ptiles[c]
            last = k == N - 1
            # rank-1 update of the upper triangle (rect cover), skipping the
            # pivot row and column (they are set exactly by pivot_head);
            # returns emission closures so the scheduler can interleave the
            # two in-flight chunks' pieces
            pieces = []
            t_p = tp_pool.tile([P, MB * max_rect], F32, tag="tp")
            upd_eng = UPD_ENG_PAR[(k + c) % 2]
            for ri, ((r0, r1), (e_out, e_sub)) in enumerate(zip(RECTS, upd_eng)):
                rowg = (
                    [(r0, k), (k + 1, r1)] if r0 <= k < r1 else [(r0, r1)]
                )
                for a, b in rowg:
                    if a >= b:
                        continue
                    # skip the pivot column only when the row group is tall
                    # enough to amortize the extra op (c3[k]=0 keeps an
                    # unsplit region correct); the last pivot's split is free
                    if r0 <= k and (b - a >= 3 or last):
                        colg = [(r0, k), (k + 1, N)]
                    else:
                        colg = [(r0, N)]
                    for ca, cb_ in colg:
                        if ca >= cb_:
                            continue
                        nr, ncl = b - a, cb_ - ca
                        if ri in TMP_SLOT:
                            tr, tc_ = TMP_SLOT[ri]
                            tv = A4[
                                :, :,
                                tr + (a - r0) : tr + (b - r0),
                                tc_ + (ca - r0) : tc_ + (cb_ - r0),
                            ]
                        else:
                            tv = t_p[:, 0 : M * (r1 - r0) * (N - r0)].rearrange(
                                "p (m i j) -> p m i j", m=M, i=r1 - r0
                            )[:, :, a - r0 : b - r0, ca - r0 : cb_ - r0]
                        cb = (
                            c3[:, :, a:b]
                            .unsqueeze(3)
                            .broadcast_to([P, M, nr, ncl])
                        )
                        crb = (
                            cr3[:, :, ca:cb_]
                            .unsqueeze(2)
                            .broadcast_to([P, M, nr, ncl])
                        )
                        reg = A4[:, :, a:b, ca:cb_]

                        def emit(tv=tv, cb=cb, crb=crb, reg=reg,
                                 e_out=e_out, e_sub=e_sub, last=last):
                            eng(e_out).tensor_mul(tv, cb, crb)
                            if last:
                                # tmp - A = -(A - tmp): folds the negate in
                                eng(e_sub).tensor_sub(reg, tv, reg)
                            else:
                                eng(e_sub).tensor_sub(reg, reg, tv)

                        pieces.append(emit)
            return pieces

        def mirror_cols(c, cols):
            _, A4 = A4s[c]
            for i in cols:
                e = eng(MIRROR_CYCLE[i % len(MIRROR_CYCLE)])
                if e is nc.scalar:
                    e.copy(A4[:, :, i + 1 :, i], A4[:, :, i, i + 1 :])
                else:
                    e.tensor_copy(A4[:, :, i + 1 :, i], A4[:, :, i, i + 1 :])

        def dma_out(c, rows_lo=0, rows_hi=N, pop=True):
            M = sizes[c]
            A = (A4s.pop(c) if pop else A4s[c])[0]
            src = A[:, 0 : M * N * N].rearrange("p (m x) -> p m x", m=M)
            dst = out_f[
                :, offs[c] * N * N : (offs[c] + M) * N * N
            ].rearrange("p (m x) -> p m x", m=M)
            nc.sync.dma_start(
                dst[:, :, rows_lo * N : rows_hi * N],
                src[:, :, rows_lo * N : rows_hi * N],
            )

        # ---- rolling pipeline ----
        # phase f: sweeps pivots 0..9 of chunk f and 10..19 of chunk f-1;
        # chunk f-2 is mirrored + DMA'd out in the early steps, then chunk
        # f+1 builds mid-phase (reusing f-2's A slot; pool bufs=3).
        HALF = N // 2
        n_rects = len(RECTS)
        build_start(0)
        for ri in range(n_rects):
            build_rect(0, ri)
        for ri in range(n_rects):
            build_add(0, ri)
        build_end(0)
        for f in range(chunks + 2):
            cur = f if f < chunks else None        # sweeping pivots 0..9
            prev = f - 1 if 0 <= f - 1 < chunks else None   # pivots 10..19
            fin = f - 2 if 0 <= f - 2 < chunks else None    # mirror + DMA
            nxt = f + 1 if f + 1 < chunks else None
            for k in range(HALF):
                if fin is not None:
                    if k == 0:
                        mirror_cols(fin, range(0, N // 2))
                    elif k == 1:
                        if fin == chunks - 1:
                            # drain tail: ship final rows while mirroring
                            dma_out(fin, 0, N // 2, pop=False)
                        mirror_cols(fin, range(N // 2, N - 1))
                    elif k == 2:
                        if fin == chunks - 1:
                            dma_out(fin, N // 2, N)
                        else:
                            dma_out(fin)
                if nxt is not None:
                    if k == 2:
                        build_start(nxt)
                        for ri in range(n_rects):
                            build_rect(nxt, ri)
                    elif k == 3:
                        for ri in range(n_rects):
                            build_add(nxt, ri)
                    elif k == 4:
                        build_end(nxt)
                if cur is not None:
                    pivot_head(cur, k)
                if prev is not None:
                    pivot_head(prev, HALF + k)
                bod_a = pivot_body(cur, k) if cur is not None else []
                bod_b = pivot_body(prev, HALF + k) if prev is not None else []
                for e in bod_a:
                    e()
                for e in bod_b:
                    e()
        _zt_free()


_CACHE = {}


def build_nc(b_core=B_CORE, m_chunk=32, num_devices=N_CORES):
    key = (b_core, m_chunk, num_devices)
    if key in _CACHE:
        return _CACHE[key]
    nc = bacc.Bacc(
        "TRN2", target_bir_lowering=False, debug=False, num_devices=num_devices
    )
    pos_d = nc.dram_tensor("pos", [b_core, N * D], F32, kind="ExternalInput")
    out_d = nc.dram_tensor("out", [b_core, N * N], F32, kind="ExternalOutput")
    with tile.TileContext(nc) as tc:
        emit_kernel(tc, pos_d.ap(), out_d.ap(), b_core, m_chunk)
    nc.compile()
    _CACHE[key] = nc
    return nc


def run(pos_full, b_core=B_CORE, m_chunk=32, n_cores=N_CORES, **kw):
    """pos_full: [n_cores*b_core, 20, 2] f32 -> [n_cores*b_core, 20, 20] f32."""
    nc = build_nc(b_core, m_chunk, n_cores)
    flat = np.ascontiguousarray(
        np.asarray(pos_full, dtype=np.float32).reshape(-1, N * D)
    )
    in_maps = [
        {"pos": flat[i * b_core : (i + 1) * b_core]} for i in range(n_cores)
    ]
    res = run_bass_kernel_spmd(nc, in_maps, core_ids=list(range(n_cores)), **kw)
    out = np.concatenate([r["out"] for r in res.results], axis=0)
    return out.reshape(-1, N, N), res


def kernel(neighbor_positions, edge_list=None):
    out, _ = run(neighbor_positions)
    return out
